# revision 1
# baseline (speedup 1.0000x reference)
"""Depthwise cross-correlation (per-sample dynamic kernel) on 8 Trainium2 cores.

reference: out[b,i,j,c] = sum_{di,dj} search[b,i+di,j+dj,c] * template[b,di,dj,c]
  search [64,31,31,256] f32, template [64,7,7,256] f32 -> out [64,25,25,256] f32

Strategy (pure data parallel, 8 samples/core, no collectives):
- Host marshals inputs channel-major: per (sample, channel-half) a single
  "blob" [128, 961 + 49*128] = flattened search | 49 host-built diagonal
  weight tiles diag(template[k]) (the only way a systolic array computes a
  depthwise product: out = diag(t_k) @ shift_k(S), accumulated in PSUM over
  the 49 taps).
- Matmuls run in float32r (fp32 bits, 2-elem/cycle PE streaming, ~1e-4 rel
  err) which requires: even innermost counts, dst partition 0, 8B alignment
  -> output window padded 25->26 cols, full-height 128 diags.
- Output rows split 13+12 so each accumulation target fits one PSUM bank
  with N>=256 (below 256 float32r drops to quarter rate).
- A post-pass splits multi-wait instructions (walrus allows one sync-wait
  per instruction) into single-wait NoOp carriers.
"""
import sys

sys.path.insert(0, "/opt/trn_rl_repo")

import numpy as np
import concourse.bass as bass
import concourse.mybir as mybir
import concourse.tile as tile
from concourse.bass_utils import run_bass_kernel_spmd

B = 64
X, K, OUT = 31, 7, 25
CH = 256
C = 128                      # channels per half (partition dim)
N_CORES = 8
BPC = B // N_CORES           # samples per core
SLEN = X * X                 # 961
DLEN = K * K * 128           # 6272
BLOB = SLEN + DLEN           # 7233
R0, R1 = 13, 12              # output row split (13*26=338, 12*26=312 cols)
W = 26                       # even output window; col 25 is padding

_CACHE = {}


def _corr_half(nc, sb, ps, blob_t, out_dram_h):
    d_v = blob_t[:, SLEN:].rearrange("c (k m) -> c k m", k=K * K)
    pa = ps.tile([C, R0, W], mybir.dt.float32, tag="pa")
    pb = ps.tile([C, R1, W], mybir.dt.float32, tag="pb")
    for (pt, r_base, nrows) in [(pa, 0, R0), (pb, R0, R1)]:
        for k in range(K * K):
            di, dj = divmod(k, K)
            off = (r_base + di) * X + dj
            rows = blob_t[:, off : off + X * nrows].rearrange(
                "c (r j) -> c r j", j=X)[:, :, 0:W]
            nc.tensor.matmul(pt[:, :, :], d_v[:, k, :], rows,
                             start=(k == 0), stop=(k == K * K - 1),
                             skip_group_check=True)
    out_sb = sb.tile([C, OUT, OUT], mybir.dt.float32, tag="out_sb")
    nc.vector.tensor_copy(out=out_sb[:, 0:R0, :], in_=pa[:, :, 0:OUT])
    nc.vector.tensor_copy(out=out_sb[:, R0:OUT, :], in_=pb[:, :, 0:OUT])
    nc.sync.dma_start(out=out_dram_h, in_=out_sb[:])


def _split_excess_waits(nc):
    """Walrus codegen allows a single sync-wait command per instruction.
    Move extra waits onto inserted same-engine NoOps; firing a monotone
    wait earlier on the same queue is always safe."""
    for fn in nc.m.functions:
        for bb in fn.blocks:
            out = []
            for inst in bb.instructions:
                si = inst.sync_info
                if si is not None and len(si.on_wait) > 1:
                    waits = list(si.on_wait)
                    for w in waits[:-1]:
                        nop = mybir.InstNoOp(
                            name=nc.get_next_instruction_name(), ins=[], outs=[])
                        nop.engine = inst.engine
                        nop.sync_info = mybir.SyncInfo(on_wait=[w], on_update=[])
                        out.append(nop)
                    si.on_wait = [waits[-1]]
                out.append(inst)
            bb.instructions = out


def _build_nc():
    nc = bass.Bass("TRN2", debug=False)
    b_in = nc.dram_tensor("blob", [BPC, 2, C, BLOB], mybir.dt.float32r,
                          kind="ExternalInput").ap()
    o_out = nc.dram_tensor("o", [BPC, 2, C, OUT, OUT], mybir.dt.float32,
                           kind="ExternalOutput").ap()
    with tile.TileContext(nc) as tc:
        with tc.tile_pool(name="sb", bufs=3) as sb, \
             tc.tile_pool(name="os", bufs=3) as osb, \
             tc.tile_pool(name="ps", bufs=2, space="PSUM") as ps:
            del osb
            for s in range(BPC):
                for h in range(2):
                    blob_t = sb.tile([C, BLOB], mybir.dt.float32r, tag="blob")
                    nc.sync.dma_start(out=blob_t[:], in_=b_in[s, h])
                    _corr_half(nc, sb, ps, blob_t, o_out[s, h])
    _split_excess_waits(nc)
    return nc


def _marshal(search, template):
    """-> blob [B, 2, C, BLOB] float32 (search rows | 49 diag weight tiles)."""
    search = np.ascontiguousarray(search, dtype=np.float32)
    template = np.ascontiguousarray(template, dtype=np.float32)
    # channel-major views: [B, 2, C, ...]
    s_cm = search.reshape(B, SLEN, 2, C).transpose(0, 2, 3, 1)  # [B,2,C,961]
    t_cm = template.reshape(B, K * K, 2, C).transpose(0, 2, 3, 1)  # [B,2,C,49]
    blob = np.zeros((B, 2, C, BLOB), np.float32)
    blob[:, :, :, :SLEN] = s_cm
    d = blob[:, :, :, SLEN:].reshape(B, 2, C, K * K, 128)
    c = np.arange(C)
    # d[b,h,c,k,c] = t_cm[b,h,c,k]
    d[:, :, c, :, c] = t_cm.transpose(2, 0, 1, 3)
    return blob


def kernel(search, template):
    if "nc" not in _CACHE:
        _CACHE["nc"] = _build_nc()
    nc = _CACHE["nc"]
    blob = _marshal(search, template)
    blob = blob.reshape(N_CORES, BPC, 2, C, BLOB)
    in_maps = [{"blob": blob[core]} for core in range(N_CORES)]
    res = run_bass_kernel_spmd(nc, in_maps, core_ids=list(range(N_CORES)))
    # o per core: [BPC, 2, C, OUT, OUT] -> [B, OUT, OUT, 256]
    o = np.stack([res.results[core]["o"] for core in range(N_CORES)])
    o = o.reshape(B, 2, C, OUT, OUT).transpose(0, 3, 4, 1, 2).reshape(B, OUT, OUT, CH)
    return np.ascontiguousarray(o)


def run_traced(search, template):
    """Like kernel() but with NTFF tracing; returns (out, BassKernelResults)."""
    if "nc" not in _CACHE:
        _CACHE["nc"] = _build_nc()
    nc = _CACHE["nc"]
    blob = _marshal(search, template).reshape(N_CORES, BPC, 2, C, BLOB)
    in_maps = [{"blob": blob[core]} for core in range(N_CORES)]
    res = run_bass_kernel_spmd(nc, in_maps, core_ids=list(range(N_CORES)),
                               trace=True)
    o = np.stack([res.results[core]["o"] for core in range(N_CORES)])
    o = o.reshape(B, 2, C, OUT, OUT).transpose(0, 3, 4, 1, 2).reshape(B, OUT, OUT, CH)
    return np.ascontiguousarray(o), res


# revision 2
# speedup vs baseline: 66.7319x; 66.7319x over previous
"""Depthwise cross-correlation (per-sample dynamic kernel) on 8 Trainium2 cores.

reference: out[b,i,j,c] = sum_{di,dj} search[b,i+di,j+dj,c] * template[b,di,dj,c]
  search [64,31,31,256] f32, template [64,7,7,256] f32 -> out [64,25,25,256] f32

Strategy (pure data parallel, 8 samples/core, no collectives):
- Host marshals inputs channel-major: per (sample, channel-half) a single
  "blob" [128, 961 + 49*128] = flattened search | 49 host-built diagonal
  weight tiles diag(template[k]) (the only way a systolic array computes a
  depthwise product: out = diag(t_k) @ shift_k(S), accumulated in PSUM over
  the 49 taps).
- Matmuls run in float32r (fp32 bits, 2-elem/cycle PE streaming, ~1e-4 rel
  err) which requires: even innermost counts, dst partition 0, 8B alignment
  -> output window padded 25->26 cols, full-height 128 diags.
- Output rows split 13+12 so each accumulation target fits one PSUM bank
  with N>=256 (below 256 float32r drops to quarter rate).
- A post-pass splits multi-wait instructions (walrus allows one sync-wait
  per instruction) into single-wait NoOp carriers.
"""
import sys

sys.path.insert(0, "/opt/trn_rl_repo")

import numpy as np
import concourse.bass as bass
import concourse.mybir as mybir
import concourse.tile as tile
from concourse.bass_utils import run_bass_kernel_spmd

B = 64
X, K, OUT = 31, 7, 25
CH = 256
C = 128                      # channels per half (partition dim)
N_CORES = 8
BPC = B // N_CORES           # samples per core
SLEN = X * X                 # 961
DLEN = K * K * 128           # 6272
BLOB = SLEN + DLEN           # 7233
R0, R1 = 13, 12              # output row split (13*26=338, 12*26=312 cols)
W = 26                       # even output window; col 25 is padding

_CACHE = {}


def _corr_half(nc, sb, ps, blob_t, out_dram_h):
    d_v = blob_t[:, SLEN:].rearrange("c (k m) -> c k m", k=K * K)
    pa = ps.tile([C, R0, W], mybir.dt.float32, tag="pa")
    pb = ps.tile([C, R1, W], mybir.dt.float32, tag="pb")
    for (pt, r_base, nrows) in [(pa, 0, R0), (pb, R0, R1)]:
        for k in range(K * K):
            di, dj = divmod(k, K)
            off = (r_base + di) * X + dj
            rows = blob_t[:, off : off + X * nrows].rearrange(
                "c (r j) -> c r j", j=X)[:, :, 0:W]
            nc.tensor.matmul(pt[:, :, :], d_v[:, k, :], rows,
                             start=(k == 0), stop=(k == K * K - 1),
                             skip_group_check=True)
    out_sb = sb.tile([C, OUT, OUT], mybir.dt.float32, tag="out_sb")
    nc.vector.tensor_copy(out=out_sb[:, 0:R0, :], in_=pa[:, :, 0:OUT])
    nc.vector.tensor_copy(out=out_sb[:, R0:OUT, :], in_=pb[:, :, 0:OUT])
    nc.sync.dma_start(out=out_dram_h, in_=out_sb[:])


def _split_excess_waits(nc):
    """Walrus codegen allows a single sync-wait command per instruction.
    Move extra waits onto inserted same-engine NoOps; firing a monotone
    wait earlier on the same queue is always safe."""
    for fn in nc.m.functions:
        for bb in fn.blocks:
            out = []
            for inst in bb.instructions:
                si = inst.sync_info
                if si is not None and len(si.on_wait) > 1:
                    waits = list(si.on_wait)
                    for w in waits[:-1]:
                        nop = mybir.InstNoOp(
                            name=nc.get_next_instruction_name(), ins=[], outs=[])
                        nop.engine = inst.engine
                        nop.sync_info = mybir.SyncInfo(on_wait=[w], on_update=[])
                        out.append(nop)
                    si.on_wait = [waits[-1]]
                out.append(inst)
            bb.instructions = out


def _build_nc(reps=1):
    nc = bass.Bass("TRN2", debug=False)
    b_in = nc.dram_tensor("blob", [BPC, 2, C, BLOB], mybir.dt.float32r,
                          kind="ExternalInput").ap()
    o_out = nc.dram_tensor("o", [BPC, 2, C, OUT, OUT], mybir.dt.float32,
                           kind="ExternalOutput").ap()
    with tile.TileContext(nc) as tc:
        with tc.tile_pool(name="sb", bufs=3) as sb, \
             tc.tile_pool(name="ps", bufs=2, space="PSUM") as ps:
            for _ in range(reps):
                for s in range(BPC):
                    for h in range(2):
                        blob_t = sb.tile([C, BLOB], mybir.dt.float32r, tag="blob")
                        nc.sync.dma_start(out=blob_t[:], in_=b_in[s, h])
                        _corr_half(nc, sb, ps, blob_t, o_out[s, h])
    _split_excess_waits(nc)
    return nc


def _marshal(search, template):
    """-> blob [B, 2, C, BLOB] float32 (search rows | 49 diag weight tiles)."""
    search = np.ascontiguousarray(search, dtype=np.float32)
    template = np.ascontiguousarray(template, dtype=np.float32)
    # channel-major views: [B, 2, C, ...]
    s_cm = search.reshape(B, SLEN, 2, C).transpose(0, 2, 3, 1)  # [B,2,C,961]
    t_cm = template.reshape(B, K * K, 2, C).transpose(0, 2, 3, 1)  # [B,2,C,49]
    blob = np.zeros((B, 2, C, BLOB), np.float32)
    blob[:, :, :, :SLEN] = s_cm
    d = blob[:, :, :, SLEN:].reshape(B, 2, C, K * K, 128)
    c = np.arange(C)
    # d[b,h,c,k,c] = t_cm[b,h,c,k]
    d[:, :, c, :, c] = t_cm.transpose(2, 0, 1, 3)
    return blob


def kernel(search, template):
    if "nc" not in _CACHE:
        _CACHE["nc"] = _build_nc()
    nc = _CACHE["nc"]
    blob = _marshal(search, template)
    blob = blob.reshape(N_CORES, BPC, 2, C, BLOB)
    in_maps = [{"blob": blob[core]} for core in range(N_CORES)]
    res = run_bass_kernel_spmd(nc, in_maps, core_ids=list(range(N_CORES)))
    # o per core: [BPC, 2, C, OUT, OUT] -> [B, OUT, OUT, 256]
    o = np.stack([res.results[core]["o"] for core in range(N_CORES)])
    o = o.reshape(B, 2, C, OUT, OUT).transpose(0, 3, 4, 1, 2).reshape(B, OUT, OUT, CH)
    return np.ascontiguousarray(o)


def run_traced(search, template):
    """Like kernel() but with NTFF tracing; returns (out, BassKernelResults)."""
    if "nc" not in _CACHE:
        _CACHE["nc"] = _build_nc()
    nc = _CACHE["nc"]
    blob = _marshal(search, template).reshape(N_CORES, BPC, 2, C, BLOB)
    in_maps = [{"blob": blob[core]} for core in range(N_CORES)]
    res = run_bass_kernel_spmd(nc, in_maps, core_ids=list(range(N_CORES)),
                               trace=True)
    o = np.stack([res.results[core]["o"] for core in range(N_CORES)])
    o = o.reshape(B, 2, C, OUT, OUT).transpose(0, 3, 4, 1, 2).reshape(B, OUT, OUT, CH)
    return np.ascontiguousarray(o), res
